# revision 1
# baseline (speedup 1.0000x reference)
"""Causal GQA self-attention (B=4, T=2048, C=2048, 16 Q heads / 8 KV heads,
hd=128) as a Bass/Tile SPMD kernel on 8 Trainium2 NeuronCores.

Sharding: core c = (batch b = c//2, head-group g = c%2). Each core handles one
batch and 8 Q heads / 4 KV heads. Wq/Wk/Wv column-sharded on the head dim, Wo
row-sharded; the host sums the two partial Wo products per batch (2-way
all-reduce done on host during the gather).

All on-device tensors live in a transposed [feature, token] layout so every
matmul contraction sits on the partition dim with no on-device transposes:
  qT/kT = [d, t], v = [t, d], scores as S^T = [k, q], output as y^T = [o, t].
Bulk matmuls run in bf16 (fp32 PSUM accumulation; ~4e-3 end-to-end rel err).
The loop is software-pipelined: attention/Wo of block tb-1 interleave with
the projections of block tb so projection matmuls fill PE gaps while the
ScalarE exp stream drains; softmax denominators accumulate on the PE via an
accumulating ones-matmul, reciprocals use the single-op approx DVE path, and
causal masking is a GpSimd memset + one [128,128] triangular multiply.
"""

import sys

import ml_dtypes
import numpy as np

sys.path.insert(0, "/opt/trn_rl_repo")

import concourse.bass as bass  # noqa: E402
import concourse.mybir as mybir  # noqa: E402
import concourse.tile as tile  # noqa: E402
from concourse import bacc  # noqa: E402
from concourse.bass_utils import run_bass_kernel_spmd  # noqa: E402

# Problem shape (hardcoded per contest contract).
B = 4
T = 2048
C = 2048
HD = 128
N_HEAD = 16
N_KV_HEAD = 8
NQH = N_HEAD // 2  # q heads per core (group)
NKV = N_KV_HEAD // 2  # kv heads per core
TB = 512  # token block
NTB = T // TB
NCT = C // 128  # contraction tiles for the projections
SCALE = 1.0 / float(np.sqrt(HD))

F32 = mybir.dt.float32
F32R = mybir.dt.float32r
BF16 = mybir.dt.bfloat16
MULT = mybir.AluOpType.mult
ADD = mybir.AluOpType.add
EXP = mybir.ActivationFunctionType.Exp


def _rope(nc, tmpp, dst, src_psum, cosb, nsinb):
    """dst = src*cos + rot_half(src)*sin, src in [d, t] layout (d partitions).

    rot_half(x)[d] = -x[d+64] for d<64, +x[d-64] for d>=64; the sign lives in
    nsinb so both halves are plain multiplies. nsinb is the sin table rotated
    by 64 partitions (nsinb[64+i] = -sin[i], nsinb[i] = sin[64+i]) so each
    tensor_tensor has equal base partitions on its two SBUF inputs (HW rule).
    """
    t0 = tmpp.tile([HD, TB], F32, tag="t0")
    nc.scalar.copy(t0[:], src_psum[:])
    nc.vector.tensor_mul(dst, t0[:], cosb[:])
    t2 = tmpp.tile([HD, TB], F32, tag="t2")
    nc.vector.tensor_mul(t2[0:64, :], t0[64:128, :], nsinb[64:128, :])
    nc.vector.tensor_mul(t2[64:128, :], t0[0:64, :], nsinb[0:64, :])
    nc.vector.scalar_tensor_tensor(dst, t2[:], 1.0, dst, op0=MULT, op1=ADD)


def build_nc():
    nc = bacc.Bacc("TRN2", target_bir_lowering=False, debug=False, num_devices=8)

    xT = nc.dram_tensor("xT", [C, T], BF16, kind="ExternalInput")
    wqT = nc.dram_tensor("wqT", [C, NQH * HD], BF16, kind="ExternalInput")
    wkT = nc.dram_tensor("wkT", [C, NKV * HD], BF16, kind="ExternalInput")
    wvT = nc.dram_tensor("wvT", [C, NKV * HD], BF16, kind="ExternalInput")
    woT = nc.dram_tensor("woT", [NQH * HD, C], BF16, kind="ExternalInput")
    cosdt = nc.dram_tensor("cosdt", [HD, T], F32, kind="ExternalInput")
    nsindt = nc.dram_tensor("nsindt", [HD, T], F32, kind="ExternalInput")
    masks = nc.dram_tensor("masks", [4, 128, TB], BF16, kind="ExternalInput")
    onescol = nc.dram_tensor("onescol", [128, 1], BF16, kind="ExternalInput")
    onesrow = nc.dram_tensor("onesrow", [1, 128], F32R, kind="ExternalInput")
    yT = nc.dram_tensor("yT", [C, T], F32, kind="ExternalOutput")

    from contextlib import ExitStack

    with ExitStack() as es:
        tc = es.enter_context(tile.TileContext(nc))
        es.enter_context(nc.allow_low_precision("fp32r attention"))
        constp = es.enter_context(tc.tile_pool(name="const", bufs=1))
        strp = es.enter_context(tc.tile_pool(name="stream", bufs=2))
        perp = es.enter_context(tc.tile_pool(name="persist", bufs=1))
        xp = es.enter_context(tc.tile_pool(name="xp", bufs=16))
        wqp = es.enter_context(tc.tile_pool(name="wq", bufs=2))
        wkp = es.enter_context(tc.tile_pool(name="wk", bufs=2))
        wvp = es.enter_context(tc.tile_pool(name="wv", bufs=2))
        wop = es.enter_context(tc.tile_pool(name="wo", bufs=3))
        qp = es.enter_context(tc.tile_pool(name="qt", bufs=16))
        outp = es.enter_context(tc.tile_pool(name="ot", bufs=8))
        tmpp = es.enter_context(tc.tile_pool(name="tmp", bufs=2))
        expp = es.enter_context(tc.tile_pool(name="exps", bufs=8))
        denp = es.enter_context(tc.tile_pool(name="den", bufs=2))
        smallp = es.enter_context(tc.tile_pool(name="small", bufs=2))
        yp = es.enter_context(tc.tile_pool(name="ysb", bufs=2))
        projp = es.enter_context(tc.tile_pool(name="pp", bufs=3, space="PSUM"))
        spsum = es.enter_context(tc.tile_pool(name="sp", bufs=3, space="PSUM"))
        opsum = es.enter_context(tc.tile_pool(name="op", bufs=2, space="PSUM"))
        if True:
            mask_t = []
            for m in range(4):
                mt = constp.tile([128, TB], BF16, tag=f"mask{m}")
                nc.sync.dma_start(mt[:], masks[m])
                mask_t.append(mt)
            ones_c = constp.tile([128, 1], BF16, tag="onesc")
            nc.sync.dma_start(ones_c[:], onescol[:])
            ones_r = constp.tile([1, 128], F32R, tag="onesr")
            nc.sync.dma_start(ones_r[:], onesrow[:])

            kT = [perp.tile([HD, T], BF16, tag=f"kT{h}", name=f"kT{h}") for h in range(NKV)]
            vT = [perp.tile([128, NKV * HD], BF16, tag=f"v{i}", name=f"v{i}") for i in range(T // 128)]

            def load_block(tb):
                tsl = slice(tb * TB, (tb + 1) * TB)
                xb = []
                for ct in range(NCT):
                    t_ = xp.tile([128, TB], BF16, tag="xb", name=f"xb{tb}_{ct}")
                    nc.sync.dma_start(t_[:], xT[ct * 128 : (ct + 1) * 128, tsl])
                    xb.append(t_)
                cosb = strp.tile([HD, TB], F32, tag="cosb", name=f"cosb{tb}")
                nc.sync.dma_start(cosb[:], cosdt[:, tsl])
                nsinb = strp.tile([HD, TB], F32, tag="nsinb", name=f"nsinb{tb}")
                nc.sync.dma_start(nsinb[:], nsindt[:, tsl])
                return xb, cosb, nsinb

            def proj_block(tb, xb, cosb, nsinb):
                tsl = slice(tb * TB, (tb + 1) * TB)
                # K projection (k^T layout [d, t]) + RoPE
                for kw in range(2):
                    kps = [projp.tile([128, TB], F32, tag="pp", name=f"kps{tb}_{kw}_{i}") for i in range(2)]
                    for ct in range(NCT):
                        wkt = wkp.tile([128, 256], BF16, tag="wk", name=f"wk{tb}_{kw}_{ct}")
                        nc.sync.dma_start(wkt[:], wkT[ct * 128 : (ct + 1) * 128, kw * 256 : (kw + 1) * 256])
                        for i in range(2):
                            nc.tensor.matmul(
                                kps[i][:],
                                wkt[:, i * 128 : (i + 1) * 128],
                                xb[ct][:],
                                start=(ct == 0),
                                stop=(ct == NCT - 1),
                            )
                    for i in range(2):
                        _rope(nc, tmpp, kT[kw * 2 + i][:, tsl], kps[i], cosb, nsinb)

                # V projection in [t, d] layout
                for vw in range(2):
                    vps = [projp.tile([128, NKV * HD], F32, tag="pp", name=f"vps{tb}_{vw}_{i}") for i in range(2)]
                    for ct in range(NCT):
                        wvt = wvp.tile([128, NKV * HD], BF16, tag="wv", name=f"wv{tb}_{vw}_{ct}")
                        nc.sync.dma_start(wvt[:], wvT[ct * 128 : (ct + 1) * 128, :])
                        for i in range(2):
                            nc.tensor.matmul(
                                vps[i][:],
                                xb[ct][:, (vw * 2 + i) * 128 : (vw * 2 + i + 1) * 128],
                                wvt[:],
                                start=(ct == 0),
                                stop=(ct == NCT - 1),
                            )
                    for i in range(2):
                        nc.vector.tensor_copy(vT[4 * tb + vw * 2 + i][:], vps[i][:])

                # Q projection (q^T layout) + RoPE, two waves of 4
                qts = []
                for wave in range(4):
                    qps = [projp.tile([128, TB], F32, tag="pp", name=f"qps{tb}_{wave}_{i}") for i in range(2)]
                    for ct in range(NCT):
                        wqt = wqp.tile([128, 256], BF16, tag="wq", name=f"wq{tb}_{wave}_{ct}")
                        nc.sync.dma_start(
                            wqt[:],
                            wqT[ct * 128 : (ct + 1) * 128, wave * 256 : (wave + 1) * 256],
                        )
                        for o in range(2):
                            nc.tensor.matmul(
                                qps[o][:],
                                wqt[:, o * 128 : (o + 1) * 128],
                                xb[ct][:],
                                start=(ct == 0),
                                stop=(ct == NCT - 1),
                            )
                    for o in range(2):
                        qt = qp.tile([HD, TB], BF16, tag="qt", name=f"qt{tb}_{wave}_{o}")
                        _rope(nc, tmpp, qt[:], qps[o], cosb, nsinb)
                        qts.append(qt)
                return qts

            def attention_block(tb, qts):
                ktmax = 4 * tb + 4
                outs = []
                tri = mask_t[0]  # [:, 0:128] is the lower-tri diagonal mask
                for h in range(NQH):
                    hv = h // 2
                    ops_ = opsum.tile([HD, TB], F32, tag="op", name=f"aop{tb}_{h}")
                    den = opsum.tile([1, TB], F32, tag="op", name=f"den{tb}_{h}")
                    for kt in range(ktmax):
                        sps = spsum.tile([128, TB], F32, tag="sp")
                        nc.tensor.matmul(
                            sps[:],
                            kT[hv][:, kt * 128 : (kt + 1) * 128],
                            qts[h][:],
                            start=True,
                            stop=True,
                        )
                        ex = expp.tile([128, TB], BF16, tag="exps")
                        nc.scalar.activation(ex[:], sps[:], EXP, scale=SCALE)
                        m = kt - 4 * tb
                        if m >= 0:
                            # causal: zero fully-masked q-subtiles (idle GpSimd)
                            # and apply the triangular mask on the diagonal one
                            if m > 0:
                                nc.gpsimd.memset(ex[:, 0 : 128 * m], 0.0)
                            nc.vector.tensor_mul(
                                ex[:, 128 * m : 128 * (m + 1)],
                                ex[:, 128 * m : 128 * (m + 1)],
                                tri[:, 0:128],
                            )
                        # denominator: accumulate ones.T @ ex on the PE in psum
                        nc.tensor.matmul(
                            den[:],
                            ones_c[:],
                            ex[:],
                            start=(kt == 0),
                            stop=(kt == ktmax - 1),
                        )
                        nc.tensor.matmul(
                            ops_[:],
                            vT[kt][:, hv * 128 : (hv + 1) * 128],
                            ex[:],
                            start=(kt == 0),
                            stop=(kt == ktmax - 1),
                        )
                    # single-op approx reciprocal (~18 bits, plenty), then
                    # partition-broadcast on the otherwise idle GpSimd engine
                    rec = smallp.tile([1, TB], F32, tag="rec")
                    nc.vector.reciprocal_approx_fast(rec[:], den[:])
                    bcs = smallp.tile([128, TB], F32, tag="bcs")
                    nc.gpsimd.partition_broadcast(bcs[:], rec[0:1, :])
                    ot = outp.tile([HD, TB], BF16, tag="ot")
                    nc.vector.tensor_mul(ot[:], ops_[:], bcs[:])
                    outs.append(ot)
                return outs

            def wo_block(tb, outs):
                tsl = slice(tb * TB, (tb + 1) * TB)
                for c2 in range(8):
                    yps = [projp.tile([128, TB], F32, tag="pp", name=f"yps{tb}_{c2}_{i}") for i in range(2)]
                    for jh in range(NQH):
                        wot = wop.tile([128, 256], BF16, tag="wo", name=f"wo{tb}_{c2}_{jh}")
                        nc.sync.dma_start(
                            wot[:],
                            woT[jh * 128 : (jh + 1) * 128, c2 * 256 : (c2 + 1) * 256],
                        )
                        for o in range(2):
                            nc.tensor.matmul(
                                yps[o][:],
                                wot[:, o * 128 : (o + 1) * 128],
                                outs[jh][:],
                                start=(jh == 0),
                                stop=(jh == NQH - 1),
                            )
                    for o in range(2):
                        ysb = yp.tile([128, TB], F32, tag="ysb")
                        nc.scalar.copy(ysb[:], yps[o][:])
                        og = c2 * 2 + o
                        nc.sync.dma_start(yT[og * 128 : (og + 1) * 128, tsl], ysb[:])

            # Software pipeline: attention/Wo of block tb-1 are emitted BEFORE
            # the projections of block tb, so the ACT-gated attention phase
            # always has dense projection matmuls to fill PE gaps (keeps the
            # HAM clock gate warm).
            prev_qts = None
            for tb in range(NTB):
                xb, cosb, nsinb = load_block(tb)
                if prev_qts is not None:
                    outs = attention_block(tb - 1, prev_qts)
                    wo_block(tb - 1, outs)
                prev_qts = proj_block(tb, xb, cosb, nsinb)
            outs = attention_block(NTB - 1, prev_qts)
            wo_block(NTB - 1, outs)

    nc.compile()
    return nc


def _host_consts():
    inv_freq = 1.0 / (10000.0 ** (np.arange(0, HD, 2, dtype=np.float32) / HD))
    t = np.arange(T, dtype=np.float32)
    freqs = np.outer(t, inv_freq)  # [T, HD/2]
    freqs = np.repeat(freqs, 2, axis=-1)  # [T, HD]
    cos = np.cos(freqs).astype(np.float32).T.copy()  # [HD, T]
    sin = np.sin(freqs).astype(np.float32).T.copy()
    # rotated-by-64 signed sin table: row d holds the multiplier that pairs
    # with x[(d+64)%128]; rows 64..127 carry -sin[0:64], rows 0..63 +sin[64:128]
    nsin = np.empty_like(sin)
    nsin[0:64, :] = sin[64:128, :]
    nsin[64:128, :] = -sin[0:64, :]

    masks = np.zeros((4, 128, TB), dtype=ml_dtypes.bfloat16)
    kp = np.arange(128)[:, None]
    qf = np.arange(TB)[None, :]
    for m in range(4):
        vis = (qf // 128 > m) | ((qf // 128 == m) & (kp <= qf % 128))
        masks[m] = vis.astype(ml_dtypes.bfloat16)

    return {
        "cosdt": np.ascontiguousarray(cos),
        "nsindt": np.ascontiguousarray(nsin),
        "masks": masks,
        "onescol": np.ones((128, 1), dtype=ml_dtypes.bfloat16),
        "onesrow": np.ones((1, 128), dtype=np.float32),
    }


_NC_CACHE = None


def _get_nc():
    global _NC_CACHE
    if _NC_CACHE is None:
        _NC_CACHE = build_nc()
    return _NC_CACHE


def kernel(x, Wq, Wk, Wv, Wo, _trace=False):
    x = np.asarray(x, dtype=np.float32)
    Wq = np.asarray(Wq, dtype=np.float32)
    Wk = np.asarray(Wk, dtype=np.float32)
    Wv = np.asarray(Wv, dtype=np.float32)
    Wo = np.asarray(Wo, dtype=np.float32)

    nc = _get_nc()
    consts = _host_consts()

    bf = ml_dtypes.bfloat16
    xTs = [np.ascontiguousarray(x[b].T.astype(bf)) for b in range(B)]
    wqTs = [np.ascontiguousarray(Wq[1024 * g : 1024 * (g + 1), :].T.astype(bf)) for g in range(2)]
    wkTs = [np.ascontiguousarray(Wk[512 * g : 512 * (g + 1), :].T.astype(bf)) for g in range(2)]
    wvTs = [np.ascontiguousarray(Wv[512 * g : 512 * (g + 1), :].T.astype(bf)) for g in range(2)]
    woTs = [np.ascontiguousarray(Wo[:, 1024 * g : 1024 * (g + 1)].T.astype(bf)) for g in range(2)]

    in_maps = []
    for c in range(8):
        b, g = c // 2, c % 2
        im = {
            "xT": xTs[b],
            "wqT": wqTs[g],
            "wkT": wkTs[g],
            "wvT": wvTs[g],
            "woT": woTs[g],
        }
        im.update(consts)
        in_maps.append(im)

    res = run_bass_kernel_spmd(nc, in_maps, core_ids=list(range(8)), trace=_trace)

    y = np.empty((B, T, C), dtype=np.float32)
    for b in range(B):
        y[b] = (res.results[2 * b]["yT"] + res.results[2 * b + 1]["yT"]).T
    if _trace:
        return y, res
    return y



# revision 5
# speedup vs baseline: 1.9138x; 1.9138x over previous
"""Causal GQA self-attention (B=4, T=2048, C=2048, 16 Q heads / 8 KV heads,
hd=128) as a Bass/Tile SPMD kernel on 8 Trainium2 NeuronCores.

Sharding: core c = (batch b = c//2, head-group g = c%2). Each core handles one
batch and 8 Q heads / 4 KV heads. Wq/Wk/Wv column-sharded on the head dim, Wo
row-sharded; the host sums the two partial Wo products per batch (2-way
all-reduce done on host during the gather).

All on-device tensors live in a transposed [feature, token] layout so every
matmul contraction sits on the partition dim with no on-device transposes:
  qT/kT = [d, t], v = [t, d], scores as S^T = [k, q], output as y^T = [o, t].
Bulk matmuls run in bf16 (fp32 PSUM accumulation).

v2 scheduling notes (the Tile scheduler is a run-ahead list scheduler: each
engine pops the lowest-emission-priority READY instruction, so gap-filling
across the attention/projection streams is automatic IF inputs and slots are
available):
  - all weights + rope tables are SBUF-persistent, loaded once up front, so
    projection matmuls are never gated on mid-kernel weight DMA (the v1
    bottleneck: HAM clock-gate oscillation from PE starvation).
  - x is streamed per 512-token block through a deep pool so block tb+1's
    tiles land while block tb is consumed.
  - RoPE reads the projection PSUM directly on the DVE (no ScalarE copy);
    ScalarE does (almost) nothing but the softmax exp stream.
  - causally dead columns of diagonal score tiles are never computed: the
    score/exp/den/out ops are sliced to [128*m:] instead of masked+zeroed.
  - softmax denominators accumulate on the PE via an accumulating
    ones-matmul; reciprocals use the single-op approx DVE path; the
    partition broadcast runs on the otherwise idle GpSimd.
"""

import sys

import ml_dtypes
import numpy as np

sys.path.insert(0, "/opt/trn_rl_repo")

import concourse.bass as bass  # noqa: E402
import concourse.mybir as mybir  # noqa: E402
import concourse.tile as tile  # noqa: E402
from concourse import bacc  # noqa: E402
from concourse.bass_utils import run_bass_kernel_spmd  # noqa: E402

# Problem shape (hardcoded per contest contract).
B = 4
T = 2048
C = 2048
HD = 128
N_HEAD = 16
N_KV_HEAD = 8
NQH = N_HEAD // 2  # q heads per core (group)
NKV = N_KV_HEAD // 2  # kv heads per core
TB = 512  # token block
NTB = T // TB
NCT = C // 128  # contraction tiles for the projections
SCALE = 1.0 / float(np.sqrt(HD))

F32 = mybir.dt.float32
BF16 = mybir.dt.bfloat16
MULT = mybir.AluOpType.mult
ADD = mybir.AluOpType.add
EXP = mybir.ActivationFunctionType.Exp


def build_nc():
    nc = bacc.Bacc("TRN2", target_bir_lowering=False, debug=False, num_devices=8)

    xT = nc.dram_tensor("xT", [C, T], BF16, kind="ExternalInput")
    wqT = nc.dram_tensor("wqT", [C, NQH * HD], BF16, kind="ExternalInput")
    wkT = nc.dram_tensor("wkT", [C, NKV * HD], BF16, kind="ExternalInput")
    wvT = nc.dram_tensor("wvT", [C, NKV * HD], BF16, kind="ExternalInput")
    woT = nc.dram_tensor("woT", [NQH * HD, C], BF16, kind="ExternalInput")
    cosdt = nc.dram_tensor("cosdt", [HD, T], BF16, kind="ExternalInput")
    nsindt = nc.dram_tensor("nsindt", [HD, T], BF16, kind="ExternalInput")
    tridt = nc.dram_tensor("tridt", [128, 128], BF16, kind="ExternalInput")
    onescol = nc.dram_tensor("onescol", [128, 1], BF16, kind="ExternalInput")
    yT = nc.dram_tensor("yT", [C, T], F32, kind="ExternalOutput")

    from contextlib import ExitStack

    with ExitStack() as es:
        tc = es.enter_context(tile.TileContext(nc))
        es.enter_context(nc.allow_low_precision("fp32r attention"))
        constp = es.enter_context(tc.tile_pool(name="const", bufs=1))
        wgtp = es.enter_context(tc.tile_pool(name="wgt", bufs=1))
        perp = es.enter_context(tc.tile_pool(name="persist", bufs=1))
        xp = es.enter_context(tc.tile_pool(name="xp", bufs=19))
        qp = es.enter_context(tc.tile_pool(name="qt", bufs=16))
        outp = es.enter_context(tc.tile_pool(name="ot", bufs=10))
        tmpp = es.enter_context(tc.tile_pool(name="tmp", bufs=2))
        expp = es.enter_context(tc.tile_pool(name="exps", bufs=6))
        smallp = es.enter_context(tc.tile_pool(name="small", bufs=2))
        yp = es.enter_context(tc.tile_pool(name="ysb", bufs=2))
        projp = es.enter_context(tc.tile_pool(name="pp", bufs=2, space="PSUM"))
        spsum = es.enter_context(tc.tile_pool(name="sp", bufs=3, space="PSUM"))
        opsum = es.enter_context(tc.tile_pool(name="op", bufs=2, space="PSUM"))
        wops = es.enter_context(tc.tile_pool(name="wop", bufs=1, space="PSUM"))

        # ---- one-time loads: consts, rope tables, all weights ----
        tri = constp.tile([128, 128], BF16, tag="tri")
        nc.sync.dma_start(tri[:], tridt[:])
        ones_c = constp.tile([128, 1], BF16, tag="onesc")
        nc.sync.dma_start(ones_c[:], onescol[:])
        cos_sb = constp.tile([HD, T], BF16, tag="cos")
        nc.sync.dma_start(cos_sb[:], cosdt[:])
        nsin_sb = constp.tile([HD, T], BF16, tag="nsin")
        nc.sync.dma_start(nsin_sb[:], nsindt[:])

        wk_sb, wq_sb, wv_sb, wo_sb = [], [], [], []
        for ct in range(NCT):
            t_ = wgtp.tile([128, NKV * HD], BF16, tag=f"wk{ct}")
            nc.sync.dma_start(t_[:], wkT[ct * 128 : (ct + 1) * 128, :])
            wk_sb.append(t_)
        for ct in range(NCT):
            t_ = wgtp.tile([128, NQH * HD], BF16, tag=f"wq{ct}")
            nc.sync.dma_start(t_[:], wqT[ct * 128 : (ct + 1) * 128, :])
            wq_sb.append(t_)
        for ct in range(NCT):
            t_ = wgtp.tile([128, NKV * HD], BF16, tag=f"wv{ct}")
            nc.sync.dma_start(t_[:], wvT[ct * 128 : (ct + 1) * 128, :])
            wv_sb.append(t_)
        for jh in range(NQH):
            t_ = wgtp.tile([128, C], BF16, tag=f"wo{jh}")
            nc.sync.dma_start(t_[:], woT[jh * 128 : (jh + 1) * 128, :])
            wo_sb.append(t_)

        kT = [perp.tile([HD, T], BF16, tag=f"kT{h}", name=f"kT{h}") for h in range(NKV)]
        vT = [perp.tile([128, NKV * HD], BF16, tag=f"v{i}", name=f"v{i}") for i in range(T // 128)]

        def _rope(dst, src_psum, tsl):
            """dst = src*cos + rot_half(src)*sin, src in [d, t] psum layout.

            rot_half(x)[d] = -x[d+64] for d<64, +x[d-64] for d>=64; the sign
            lives in nsin (rotated by 64 partitions on host) so both halves
            are plain multiplies with equal input base partitions.
            """
            cosb = cos_sb[:, tsl]
            nsinb = nsin_sb[:, tsl]
            tcc = tmpp.tile([HD, TB], F32, tag="tc")
            nc.vector.tensor_mul(tcc[:], src_psum[:], cosb)
            t2 = tmpp.tile([HD, TB], F32, tag="t2")
            nc.vector.tensor_mul(t2[0:64, :], src_psum[64:128, :], nsinb[64:128, :])
            nc.vector.tensor_mul(t2[64:128, :], src_psum[0:64, :], nsinb[0:64, :])
            nc.vector.scalar_tensor_tensor(dst, tcc[:], 1.0, t2[:], op0=MULT, op1=ADD)

        def load_block(tb):
            tsl = slice(tb * TB, (tb + 1) * TB)
            xb = []
            for ct in range(NCT):
                t_ = xp.tile([128, TB], BF16, tag="xb", name=f"xb{tb}_{ct}")
                nc.sync.dma_start(t_[:], xT[ct * 128 : (ct + 1) * 128, tsl])
                xb.append(t_)
            return xb

        def proj_block(tb, xb):
            tsl = slice(tb * TB, (tb + 1) * TB)
            # K projection (k^T layout [d, t]) + RoPE
            for kw in range(2):
                kps = [projp.tile([128, TB], F32, tag="pp", name=f"kps{tb}_{kw}_{i}") for i in range(2)]
                for ct in range(NCT):
                    for i in range(2):
                        nc.tensor.matmul(
                            kps[i][:],
                            wk_sb[ct][:, kw * 256 + i * 128 : kw * 256 + (i + 1) * 128],
                            xb[ct][:],
                            start=(ct == 0),
                            stop=(ct == NCT - 1),
                        )
                for i in range(2):
                    _rope(kT[kw * 2 + i][:, tsl], kps[i], tsl)

            # Q projection (q^T layout) + RoPE, four waves of 2
            qts = []
            for wave in range(4):
                qps = [projp.tile([128, TB], F32, tag="pp", name=f"qps{tb}_{wave}_{i}") for i in range(2)]
                for ct in range(NCT):
                    for o in range(2):
                        nc.tensor.matmul(
                            qps[o][:],
                            wq_sb[ct][:, wave * 256 + o * 128 : wave * 256 + (o + 1) * 128],
                            xb[ct][:],
                            start=(ct == 0),
                            stop=(ct == NCT - 1),
                        )
                for o in range(2):
                    qt = qp.tile([HD, TB], BF16, tag="qt", name=f"qt{tb}_{wave}_{o}")
                    _rope(qt[:], qps[o], tsl)
                    qts.append(qt)

            # V projection in [t, d] layout (x slice is the stationary side)
            for vw in range(2):
                vps = [projp.tile([128, NKV * HD], F32, tag="pp", name=f"vps{tb}_{vw}_{i}") for i in range(2)]
                for ct in range(NCT):
                    for i in range(2):
                        nc.tensor.matmul(
                            vps[i][:],
                            xb[ct][:, (vw * 2 + i) * 128 : (vw * 2 + i + 1) * 128],
                            wv_sb[ct][:],
                            start=(ct == 0),
                            stop=(ct == NCT - 1),
                        )
                for i in range(2):
                    nc.vector.tensor_copy(vT[4 * tb + vw * 2 + i][:], vps[i][:])
            return qts

        def attention_block(tb, qts):
            ktmax = 4 * tb + 4
            outs = []
            for h in range(NQH):
                hv = h // 2
                ops_ = opsum.tile([HD, TB], F32, tag="op", name=f"aop{tb}_{h}")
                den = opsum.tile([1, TB], F32, tag="op", name=f"den{tb}_{h}")
                for kt in range(ktmax):
                    m = kt - 4 * tb
                    lo = 128 * max(m, 0)  # first causally-visible q column
                    sps = spsum.tile([128, TB], F32, tag="sp")
                    nc.tensor.matmul(
                        sps[:, lo:TB],
                        kT[hv][:, kt * 128 : (kt + 1) * 128],
                        qts[h][:, lo:TB],
                        start=True,
                        stop=True,
                    )
                    ex = expp.tile([128, TB], BF16, tag="exps")
                    nc.scalar.activation(ex[:, lo:TB], sps[:, lo:TB], EXP, scale=SCALE)
                    if m >= 0:
                        # triangular mask on the diagonal 128x128 sub-tile
                        nc.vector.tensor_mul(
                            ex[:, lo : lo + 128],
                            ex[:, lo : lo + 128],
                            tri[:],
                        )
                    # denominator: accumulate ones.T @ ex on the PE in psum
                    nc.tensor.matmul(
                        den[0:1, lo:TB],
                        ones_c[:],
                        ex[:, lo:TB],
                        start=(kt == 0),
                        stop=(kt == ktmax - 1),
                    )
                    nc.tensor.matmul(
                        ops_[:, lo:TB],
                        vT[kt][:, hv * 128 : (hv + 1) * 128],
                        ex[:, lo:TB],
                        start=(kt == 0),
                        stop=(kt == ktmax - 1),
                    )
                # single-op approx reciprocal (~18 bits, plenty), then
                # partition-broadcast on the otherwise idle GpSimd engine
                rec = smallp.tile([1, TB], F32, tag="rec")
                nc.vector.reciprocal_approx_fast(rec[:], den[0:1, :])
                bcs = smallp.tile([128, TB], F32, tag="bcs")
                nc.gpsimd.partition_broadcast(bcs[:], rec[0:1, :])
                ot = outp.tile([HD, TB], BF16, tag="ot")
                nc.vector.tensor_mul(ot[:], ops_[:], bcs[:])
                outs.append(ot)
            return outs

        def wo_block(tb, outs):
            tsl = slice(tb * TB, (tb + 1) * TB)
            for og in range(16):
                yps = wops.tile([128, TB], F32, tag="wop", name=f"yps{tb}_{og}")
                for jh in range(NQH):
                    nc.tensor.matmul(
                        yps[:],
                        wo_sb[jh][:, og * 128 : (og + 1) * 128],
                        outs[jh][:],
                        start=(jh == 0),
                        stop=(jh == NQH - 1),
                    )
                ysb = yp.tile([128, TB], F32, tag="ysb")
                nc.vector.tensor_copy(ysb[:], yps[:])
                nc.gpsimd.dma_start(yT[og * 128 : (og + 1) * 128, tsl], ysb[:])

        # Software pipeline: attention/Wo of block tb-1 are emitted BEFORE
        # the projections of block tb; with run-ahead scheduling the dense
        # projection matmuls fill PE gaps in the ACT-gated attention phase.
        prev_qts = None
        for tb in range(NTB):
            xb = load_block(tb)
            if prev_qts is not None:
                outs = attention_block(tb - 1, prev_qts)
                wo_block(tb - 1, outs)
            prev_qts = proj_block(tb, xb)
        outs = attention_block(NTB - 1, prev_qts)
        wo_block(NTB - 1, outs)

    nc.compile()
    return nc


def _host_consts():
    inv_freq = 1.0 / (10000.0 ** (np.arange(0, HD, 2, dtype=np.float32) / HD))
    t = np.arange(T, dtype=np.float32)
    freqs = np.outer(t, inv_freq)  # [T, HD/2]
    freqs = np.repeat(freqs, 2, axis=-1)  # [T, HD]
    cos = np.cos(freqs).astype(np.float32).T.copy()  # [HD, T]
    sin = np.sin(freqs).astype(np.float32).T.copy()
    # rotated-by-64 signed sin table: row d holds the multiplier that pairs
    # with x[(d+64)%128]; rows 64..127 carry -sin[0:64], rows 0..63 +sin[64:128]
    nsin = np.empty_like(sin)
    nsin[0:64, :] = sin[64:128, :]
    nsin[64:128, :] = -sin[0:64, :]

    kp = np.arange(128)[:, None]
    qf = np.arange(128)[None, :]
    tri = (kp <= qf).astype(ml_dtypes.bfloat16)

    return {
        "cosdt": np.ascontiguousarray(cos.astype(ml_dtypes.bfloat16)),
        "nsindt": np.ascontiguousarray(nsin.astype(ml_dtypes.bfloat16)),
        "tridt": np.ascontiguousarray(tri),
        "onescol": np.ones((128, 1), dtype=ml_dtypes.bfloat16),
    }


_NC_CACHE = None


def _get_nc():
    global _NC_CACHE
    if _NC_CACHE is None:
        _NC_CACHE = build_nc()
    return _NC_CACHE


def kernel(x, Wq, Wk, Wv, Wo, _trace=False):
    x = np.asarray(x, dtype=np.float32)
    Wq = np.asarray(Wq, dtype=np.float32)
    Wk = np.asarray(Wk, dtype=np.float32)
    Wv = np.asarray(Wv, dtype=np.float32)
    Wo = np.asarray(Wo, dtype=np.float32)

    nc = _get_nc()
    consts = _host_consts()

    bf = ml_dtypes.bfloat16
    xTs = [np.ascontiguousarray(x[b].T.astype(bf)) for b in range(B)]
    wqTs = [np.ascontiguousarray(Wq[1024 * g : 1024 * (g + 1), :].T.astype(bf)) for g in range(2)]
    wkTs = [np.ascontiguousarray(Wk[512 * g : 512 * (g + 1), :].T.astype(bf)) for g in range(2)]
    wvTs = [np.ascontiguousarray(Wv[512 * g : 512 * (g + 1), :].T.astype(bf)) for g in range(2)]
    woTs = [np.ascontiguousarray(Wo[:, 1024 * g : 1024 * (g + 1)].T.astype(bf)) for g in range(2)]

    in_maps = []
    for c in range(8):
        b, g = c // 2, c % 2
        im = {
            "xT": xTs[b],
            "wqT": wqTs[g],
            "wkT": wkTs[g],
            "wvT": wvTs[g],
            "woT": woTs[g],
        }
        im.update(consts)
        in_maps.append(im)

    res = run_bass_kernel_spmd(nc, in_maps, core_ids=list(range(8)), trace=_trace)

    y = np.empty((B, T, C), dtype=np.float32)
    for b in range(B):
        y[b] = (res.results[2 * b]["yT"] + res.results[2 * b + 1]["yT"]).T
    if _trace:
        return y, res
    return y


# revision 6
# speedup vs baseline: 2.0070x; 1.0487x over previous
"""Causal GQA self-attention (B=4, T=2048, C=2048, 16 Q heads / 8 KV heads,
hd=128) as a Bass/Tile SPMD kernel on 8 Trainium2 NeuronCores.

Sharding: core c = (batch b = c//2, head-group g = c%2). Each core handles one
batch and 8 Q heads / 4 KV heads. Wq/Wk/Wv column-sharded on the head dim, Wo
row-sharded; the host sums the two partial Wo products per batch (2-way
all-reduce done on host during the gather).

All on-device tensors live in a transposed [feature, token] layout so every
matmul contraction sits on the partition dim with no on-device transposes:
  qT/kT = [d, t], v = [t, d], scores as S^T = [k, q], output as y^T = [o, t].
Bulk matmuls run in bf16 (fp32 PSUM accumulation).

v2 scheduling notes (the Tile scheduler is a run-ahead list scheduler: each
engine pops the lowest-emission-priority READY instruction, so gap-filling
across the attention/projection streams is automatic IF inputs and slots are
available):
  - all weights + rope tables are SBUF-persistent, loaded once up front, so
    projection matmuls are never gated on mid-kernel weight DMA (the v1
    bottleneck: HAM clock-gate oscillation from PE starvation).
  - x is streamed per 512-token block through a deep pool so block tb+1's
    tiles land while block tb is consumed.
  - RoPE reads the projection PSUM directly on the DVE (no ScalarE copy);
    ScalarE does (almost) nothing but the softmax exp stream.
  - causally dead columns of diagonal score tiles are never computed: the
    score/exp/den/out ops are sliced to [128*m:] instead of masked+zeroed.
  - softmax denominators accumulate on the PE via an accumulating
    ones-matmul; reciprocals use the single-op approx DVE path; the
    partition broadcast runs on the otherwise idle GpSimd.
"""

import sys

import ml_dtypes
import numpy as np

sys.path.insert(0, "/opt/trn_rl_repo")

import concourse.bass as bass  # noqa: E402
import concourse.mybir as mybir  # noqa: E402
import concourse.tile as tile  # noqa: E402
from concourse import bacc  # noqa: E402
from concourse.bass_utils import run_bass_kernel_spmd  # noqa: E402

# Problem shape (hardcoded per contest contract).
B = 4
T = 2048
C = 2048
HD = 128
N_HEAD = 16
N_KV_HEAD = 8
NQH = N_HEAD // 2  # q heads per core (group)
NKV = N_KV_HEAD // 2  # kv heads per core
TB = 512  # token block
NTB = T // TB
NCT = C // 128  # contraction tiles for the projections
SCALE = 1.0 / float(np.sqrt(HD))

F32 = mybir.dt.float32
BF16 = mybir.dt.bfloat16
MULT = mybir.AluOpType.mult
ADD = mybir.AluOpType.add
EXP = mybir.ActivationFunctionType.Exp


def build_nc():
    nc = bacc.Bacc("TRN2", target_bir_lowering=False, debug=False, num_devices=8)

    xT = nc.dram_tensor("xT", [C, T], BF16, kind="ExternalInput")
    wqT = nc.dram_tensor("wqT", [C, NQH * HD], BF16, kind="ExternalInput")
    wkT = nc.dram_tensor("wkT", [C, NKV * HD], BF16, kind="ExternalInput")
    wvT = nc.dram_tensor("wvT", [C, NKV * HD], BF16, kind="ExternalInput")
    woT = nc.dram_tensor("woT", [NQH * HD, C], BF16, kind="ExternalInput")
    cosdt = nc.dram_tensor("cosdt", [HD, T], BF16, kind="ExternalInput")
    nsindt = nc.dram_tensor("nsindt", [HD, T], BF16, kind="ExternalInput")
    tridt = nc.dram_tensor("tridt", [128, 128], BF16, kind="ExternalInput")
    onescol = nc.dram_tensor("onescol", [128, 1], BF16, kind="ExternalInput")
    yT = nc.dram_tensor("yT", [C, T], F32, kind="ExternalOutput")

    from contextlib import ExitStack

    with ExitStack() as es:
        tc = es.enter_context(tile.TileContext(nc))
        es.enter_context(nc.allow_low_precision("fp32r attention"))
        constp = es.enter_context(tc.tile_pool(name="const", bufs=1))
        wgtp = es.enter_context(tc.tile_pool(name="wgt", bufs=1))
        perp = es.enter_context(tc.tile_pool(name="persist", bufs=1))
        xp = es.enter_context(tc.tile_pool(name="xp", bufs=17))
        qp = es.enter_context(tc.tile_pool(name="qt", bufs=16))
        outp = es.enter_context(tc.tile_pool(name="ot", bufs=10))
        tmpp = es.enter_context(tc.tile_pool(name="tmp", bufs=2))
        expp = es.enter_context(tc.tile_pool(name="exps", bufs=12))
        smallp = es.enter_context(tc.tile_pool(name="small", bufs=2))
        yp = es.enter_context(tc.tile_pool(name="ysb", bufs=2))
        projp = es.enter_context(tc.tile_pool(name="pp", bufs=2, space="PSUM"))
        spsum = es.enter_context(tc.tile_pool(name="sp", bufs=3, space="PSUM"))
        opsum = es.enter_context(tc.tile_pool(name="op", bufs=2, space="PSUM"))
        wops = es.enter_context(tc.tile_pool(name="wop", bufs=1, space="PSUM"))

        # ---- one-time loads: consts, rope tables, all weights ----
        tri = constp.tile([128, 128], BF16, tag="tri")
        nc.sync.dma_start(tri[:], tridt[:])
        ones_c = constp.tile([128, 1], BF16, tag="onesc")
        nc.sync.dma_start(ones_c[:], onescol[:])
        cos_sb = constp.tile([HD, T], BF16, tag="cos")
        nc.sync.dma_start(cos_sb[:], cosdt[:])
        nsin_sb = constp.tile([HD, T], BF16, tag="nsin")
        nc.sync.dma_start(nsin_sb[:], nsindt[:])

        # x(0) is emitted before the weights so the first projection block
        # is not queued behind 12MB of weight DMA on the sync queue.
        xb0 = []
        for ct in range(NCT):
            t_ = xp.tile([128, TB], BF16, tag="xb", name=f"xb0_{ct}")
            nc.sync.dma_start(t_[:], xT[ct * 128 : (ct + 1) * 128, 0:TB])
            xb0.append(t_)

        wk_sb, wq_sb, wv_sb, wo_sb = [], [], [], []
        for ct in range(NCT):
            t_ = wgtp.tile([128, NKV * HD], BF16, tag=f"wk{ct}")
            nc.sync.dma_start(t_[:], wkT[ct * 128 : (ct + 1) * 128, :])
            wk_sb.append(t_)
        for ct in range(NCT):
            t_ = wgtp.tile([128, NQH * HD], BF16, tag=f"wq{ct}")
            nc.sync.dma_start(t_[:], wqT[ct * 128 : (ct + 1) * 128, :])
            wq_sb.append(t_)
        for ct in range(NCT):
            t_ = wgtp.tile([128, NKV * HD], BF16, tag=f"wv{ct}")
            nc.sync.dma_start(t_[:], wvT[ct * 128 : (ct + 1) * 128, :])
            wv_sb.append(t_)
        for jh in range(NQH):
            t_ = wgtp.tile([128, C], BF16, tag=f"wo{jh}")
            nc.sync.dma_start(t_[:], woT[jh * 128 : (jh + 1) * 128, :])
            wo_sb.append(t_)

        kT = [perp.tile([HD, T], BF16, tag=f"kT{h}", name=f"kT{h}") for h in range(NKV)]
        vT = [perp.tile([128, NKV * HD], BF16, tag=f"v{i}", name=f"v{i}") for i in range(T // 128)]

        def _rope(dst, src_psum, tsl):
            """dst = src*cos + rot_half(src)*sin, src in [d, t] psum layout.

            rot_half(x)[d] = -x[d+64] for d<64, +x[d-64] for d>=64; the sign
            lives in nsin (rotated by 64 partitions on host) so both halves
            are plain multiplies with equal input base partitions.
            """
            cosb = cos_sb[:, tsl]
            nsinb = nsin_sb[:, tsl]
            tcc = tmpp.tile([HD, TB], BF16, tag="tc")
            nc.vector.tensor_mul(tcc[:], src_psum[:], cosb)
            t2 = tmpp.tile([HD, TB], BF16, tag="t2")
            nc.vector.tensor_mul(t2[0:64, :], src_psum[64:128, :], nsinb[64:128, :])
            nc.vector.tensor_mul(t2[64:128, :], src_psum[0:64, :], nsinb[0:64, :])
            nc.vector.scalar_tensor_tensor(dst, tcc[:], 1.0, t2[:], op0=MULT, op1=ADD)

        def load_block(tb):
            tsl = slice(tb * TB, (tb + 1) * TB)
            xb = []
            for ct in range(NCT):
                t_ = xp.tile([128, TB], BF16, tag="xb", name=f"xb{tb}_{ct}")
                nc.sync.dma_start(t_[:], xT[ct * 128 : (ct + 1) * 128, tsl])
                xb.append(t_)
            return xb

        def proj_block(tb, xb):
            tsl = slice(tb * TB, (tb + 1) * TB)
            # K projection (k^T layout [d, t]) + RoPE
            for kw in range(2):
                kps = [projp.tile([128, TB], F32, tag="pp", name=f"kps{tb}_{kw}_{i}") for i in range(2)]
                for ct in range(NCT):
                    for i in range(2):
                        nc.tensor.matmul(
                            kps[i][:],
                            wk_sb[ct][:, kw * 256 + i * 128 : kw * 256 + (i + 1) * 128],
                            xb[ct][:],
                            start=(ct == 0),
                            stop=(ct == NCT - 1),
                        )
                for i in range(2):
                    _rope(kT[kw * 2 + i][:, tsl], kps[i], tsl)

            # Q projection (q^T layout) + RoPE, four waves of 2
            qts = []
            for wave in range(4):
                qps = [projp.tile([128, TB], F32, tag="pp", name=f"qps{tb}_{wave}_{i}") for i in range(2)]
                for ct in range(NCT):
                    for o in range(2):
                        nc.tensor.matmul(
                            qps[o][:],
                            wq_sb[ct][:, wave * 256 + o * 128 : wave * 256 + (o + 1) * 128],
                            xb[ct][:],
                            start=(ct == 0),
                            stop=(ct == NCT - 1),
                        )
                for o in range(2):
                    qt = qp.tile([HD, TB], BF16, tag="qt", name=f"qt{tb}_{wave}_{o}")
                    _rope(qt[:], qps[o], tsl)
                    qts.append(qt)

            # V projection in [t, d] layout (x slice is the stationary side)
            for vw in range(2):
                vps = [projp.tile([128, NKV * HD], F32, tag="pp", name=f"vps{tb}_{vw}_{i}") for i in range(2)]
                for ct in range(NCT):
                    for i in range(2):
                        nc.tensor.matmul(
                            vps[i][:],
                            xb[ct][:, (vw * 2 + i) * 128 : (vw * 2 + i + 1) * 128],
                            wv_sb[ct][:],
                            start=(ct == 0),
                            stop=(ct == NCT - 1),
                        )
                for i in range(2):
                    nc.scalar.copy(vT[4 * tb + vw * 2 + i][:], vps[i][:])
            return qts

        def attention_block(tb, qts):
            ktmax = 4 * tb + 4
            outs = []
            for h in range(NQH):
                hv = h // 2
                ops_ = opsum.tile([HD, TB], F32, tag="op", name=f"aop{tb}_{h}")
                den = opsum.tile([1, TB], F32, tag="op", name=f"den{tb}_{h}")
                exs = [None] * ktmax

                def emit_score(kt):
                    m = kt - 4 * tb
                    lo = 128 * max(m, 0)  # first causally-visible q column
                    sps = spsum.tile([128, TB], F32, tag="sp")
                    nc.tensor.matmul(
                        sps[:, lo:TB],
                        kT[hv][:, kt * 128 : (kt + 1) * 128],
                        qts[h][:, lo:TB],
                        start=True,
                        stop=True,
                    )
                    ex = expp.tile([128, TB], BF16, tag="exps")
                    nc.scalar.activation(ex[:, lo:TB], sps[:, lo:TB], EXP, scale=SCALE)
                    if m >= 0:
                        # triangular mask on the diagonal 128x128 sub-tile
                        nc.vector.tensor_mul(
                            ex[:, lo : lo + 128],
                            ex[:, lo : lo + 128],
                            tri[:],
                        )
                    exs[kt] = ex

                def emit_acc(kt):
                    m = kt - 4 * tb
                    lo = 128 * max(m, 0)
                    ex = exs[kt]
                    # denominator: accumulate ones.T @ ex on the PE in psum
                    nc.tensor.matmul(
                        den[0:1, lo:TB],
                        ones_c[:],
                        ex[:, lo:TB],
                        start=(kt == 0),
                        stop=(kt == ktmax - 1),
                    )
                    nc.tensor.matmul(
                        ops_[:, lo:TB],
                        vT[kt][:, hv * 128 : (hv + 1) * 128],
                        ex[:, lo:TB],
                        start=(kt == 0),
                        stop=(kt == ktmax - 1),
                    )

                # 2-step skew: score/exp run two kt ahead of den/out so the
                # accumulating matmuls never wait on the ACT exp stream.
                for kt in range(ktmax):
                    emit_score(kt)
                    if kt >= 2:
                        emit_acc(kt - 2)
                emit_acc(ktmax - 2) if ktmax >= 2 else None
                emit_acc(ktmax - 1)
                # single-op approx reciprocal (~18 bits, plenty), then
                # partition-broadcast on the otherwise idle GpSimd engine
                rec = smallp.tile([1, TB], F32, tag="rec")
                nc.vector.reciprocal_approx_fast(rec[:], den[0:1, :])
                bcs = smallp.tile([128, TB], F32, tag="bcs")
                nc.gpsimd.partition_broadcast(bcs[:], rec[0:1, :])
                ot = outp.tile([HD, TB], BF16, tag="ot")
                nc.vector.tensor_mul(ot[:], ops_[:], bcs[:])
                outs.append(ot)
            return outs

        def wo_block(tb, outs):
            tsl = slice(tb * TB, (tb + 1) * TB)
            for og in range(16):
                yps = wops.tile([128, TB], F32, tag="wop", name=f"yps{tb}_{og}")
                for jh in range(NQH):
                    nc.tensor.matmul(
                        yps[:],
                        wo_sb[jh][:, og * 128 : (og + 1) * 128],
                        outs[jh][:],
                        start=(jh == 0),
                        stop=(jh == NQH - 1),
                    )
                ysb = yp.tile([128, TB], F32, tag="ysb")
                nc.scalar.copy(ysb[:], yps[:])
                nc.gpsimd.dma_start(yT[og * 128 : (og + 1) * 128, tsl], ysb[:])

        # Software pipeline: attention/Wo of block tb-1 are emitted BEFORE
        # the projections of block tb; with run-ahead scheduling the dense
        # projection matmuls fill PE gaps in the ACT-gated attention phase.
        prev_qts = None
        for tb in range(NTB):
            xb = xb0 if tb == 0 else load_block(tb)
            if prev_qts is not None:
                outs = attention_block(tb - 1, prev_qts)
                wo_block(tb - 1, outs)
            prev_qts = proj_block(tb, xb)
        outs = attention_block(NTB - 1, prev_qts)
        wo_block(NTB - 1, outs)

    nc.compile()
    return nc


def _host_consts():
    inv_freq = 1.0 / (10000.0 ** (np.arange(0, HD, 2, dtype=np.float32) / HD))
    t = np.arange(T, dtype=np.float32)
    freqs = np.outer(t, inv_freq)  # [T, HD/2]
    freqs = np.repeat(freqs, 2, axis=-1)  # [T, HD]
    cos = np.cos(freqs).astype(np.float32).T.copy()  # [HD, T]
    sin = np.sin(freqs).astype(np.float32).T.copy()
    # rotated-by-64 signed sin table: row d holds the multiplier that pairs
    # with x[(d+64)%128]; rows 64..127 carry -sin[0:64], rows 0..63 +sin[64:128]
    nsin = np.empty_like(sin)
    nsin[0:64, :] = sin[64:128, :]
    nsin[64:128, :] = -sin[0:64, :]

    kp = np.arange(128)[:, None]
    qf = np.arange(128)[None, :]
    tri = (kp <= qf).astype(ml_dtypes.bfloat16)

    return {
        "cosdt": np.ascontiguousarray(cos.astype(ml_dtypes.bfloat16)),
        "nsindt": np.ascontiguousarray(nsin.astype(ml_dtypes.bfloat16)),
        "tridt": np.ascontiguousarray(tri),
        "onescol": np.ones((128, 1), dtype=ml_dtypes.bfloat16),
    }


_NC_CACHE = None


def _get_nc():
    global _NC_CACHE
    if _NC_CACHE is None:
        _NC_CACHE = build_nc()
    return _NC_CACHE


def kernel(x, Wq, Wk, Wv, Wo, _trace=False):
    x = np.asarray(x, dtype=np.float32)
    Wq = np.asarray(Wq, dtype=np.float32)
    Wk = np.asarray(Wk, dtype=np.float32)
    Wv = np.asarray(Wv, dtype=np.float32)
    Wo = np.asarray(Wo, dtype=np.float32)

    nc = _get_nc()
    consts = _host_consts()

    bf = ml_dtypes.bfloat16
    xTs = [np.ascontiguousarray(x[b].T.astype(bf)) for b in range(B)]
    wqTs = [np.ascontiguousarray(Wq[1024 * g : 1024 * (g + 1), :].T.astype(bf)) for g in range(2)]
    wkTs = [np.ascontiguousarray(Wk[512 * g : 512 * (g + 1), :].T.astype(bf)) for g in range(2)]
    wvTs = [np.ascontiguousarray(Wv[512 * g : 512 * (g + 1), :].T.astype(bf)) for g in range(2)]
    woTs = [np.ascontiguousarray(Wo[:, 1024 * g : 1024 * (g + 1)].T.astype(bf)) for g in range(2)]

    in_maps = []
    for c in range(8):
        b, g = c // 2, c % 2
        im = {
            "xT": xTs[b],
            "wqT": wqTs[g],
            "wkT": wkTs[g],
            "wvT": wvTs[g],
            "woT": woTs[g],
        }
        im.update(consts)
        in_maps.append(im)

    res = run_bass_kernel_spmd(nc, in_maps, core_ids=list(range(8)), trace=_trace)

    y = np.empty((B, T, C), dtype=np.float32)
    for b in range(B):
        y[b] = (res.results[2 * b]["yT"] + res.results[2 * b + 1]["yT"]).T
    if _trace:
        return y, res
    return y


# revision 8
# speedup vs baseline: 2.1195x; 1.0560x over previous
"""Causal GQA self-attention (B=4, T=2048, C=2048, 16 Q heads / 8 KV heads,
hd=128) as a Bass/Tile SPMD kernel on 8 Trainium2 NeuronCores.

Sharding: core c = (batch b = c//2, head-group g = c%2). Each core handles one
batch and 8 Q heads / 4 KV heads. Wq/Wk/Wv column-sharded on the head dim, Wo
row-sharded; the host sums the two partial Wo products per batch (2-way
all-reduce done on host during the gather).

All on-device tensors live in a transposed [feature, token] layout so every
matmul contraction sits on the partition dim with no on-device transposes:
  qT/kT = [d, t], v = [t, d], scores as S^T = [k, q], output as y^T = [o, t].
Bulk matmuls run in bf16 (fp32 PSUM accumulation).

v2 scheduling notes (the Tile scheduler is a run-ahead list scheduler: each
engine pops the lowest-emission-priority READY instruction, so gap-filling
across the attention/projection streams is automatic IF inputs and slots are
available):
  - all weights + rope tables are SBUF-persistent, loaded once up front, so
    projection matmuls are never gated on mid-kernel weight DMA (the v1
    bottleneck: HAM clock-gate oscillation from PE starvation).
  - x is streamed per 512-token block through a deep pool so block tb+1's
    tiles land while block tb is consumed.
  - RoPE reads the projection PSUM directly on the DVE (no ScalarE copy);
    ScalarE does (almost) nothing but the softmax exp stream.
  - causally dead columns of diagonal score tiles are never computed: the
    score/exp/den/out ops are sliced to [128*m:] instead of masked+zeroed.
  - softmax denominators accumulate on the PE via an accumulating
    ones-matmul; reciprocals use the single-op approx DVE path; the
    partition broadcast runs on the otherwise idle GpSimd.
"""

import sys

import ml_dtypes
import numpy as np

sys.path.insert(0, "/opt/trn_rl_repo")

import concourse.bass as bass  # noqa: E402
import concourse.mybir as mybir  # noqa: E402
import concourse.tile as tile  # noqa: E402
from concourse import bacc  # noqa: E402
from concourse.bass_utils import run_bass_kernel_spmd  # noqa: E402

# Problem shape (hardcoded per contest contract).
B = 4
T = 2048
C = 2048
HD = 128
N_HEAD = 16
N_KV_HEAD = 8
NQH = N_HEAD // 2  # q heads per core (group)
NKV = N_KV_HEAD // 2  # kv heads per core
TB = 512  # token block
NTB = T // TB
NCT = C // 128  # contraction tiles for the projections
SCALE = 1.0 / float(np.sqrt(HD))

F32 = mybir.dt.float32
BF16 = mybir.dt.bfloat16
MULT = mybir.AluOpType.mult
ADD = mybir.AluOpType.add
EXP = mybir.ActivationFunctionType.Exp


def build_nc():
    nc = bacc.Bacc("TRN2", target_bir_lowering=False, debug=False, num_devices=8)

    xT = nc.dram_tensor("xT", [C, T], BF16, kind="ExternalInput")
    wqT = nc.dram_tensor("wqT", [C, NQH * HD], BF16, kind="ExternalInput")
    wkT = nc.dram_tensor("wkT", [C, NKV * HD], BF16, kind="ExternalInput")
    wvT = nc.dram_tensor("wvT", [C, NKV * HD], BF16, kind="ExternalInput")
    woT = nc.dram_tensor("woT", [NQH * HD, C], BF16, kind="ExternalInput")
    cosdt = nc.dram_tensor("cosdt", [HD, T], BF16, kind="ExternalInput")
    nsindt = nc.dram_tensor("nsindt", [HD, T], BF16, kind="ExternalInput")
    tridt = nc.dram_tensor("tridt", [128, 128], BF16, kind="ExternalInput")
    onescol = nc.dram_tensor("onescol", [128, 1], BF16, kind="ExternalInput")
    yT = nc.dram_tensor("yT", [C, T], F32, kind="ExternalOutput")

    from contextlib import ExitStack

    with ExitStack() as es:
        tc = es.enter_context(tile.TileContext(nc))
        es.enter_context(nc.allow_low_precision("fp32r attention"))
        constp = es.enter_context(tc.tile_pool(name="const", bufs=1))
        wgtp = es.enter_context(tc.tile_pool(name="wgt", bufs=1))
        perp = es.enter_context(tc.tile_pool(name="persist", bufs=1))
        xp = es.enter_context(tc.tile_pool(name="xp", bufs=17))
        qp = es.enter_context(tc.tile_pool(name="qt", bufs=16))
        outp = es.enter_context(tc.tile_pool(name="ot", bufs=10))
        tmpp = es.enter_context(tc.tile_pool(name="tmp", bufs=2))
        expp = es.enter_context(tc.tile_pool(name="exps", bufs=10))
        smallp = es.enter_context(tc.tile_pool(name="small", bufs=2))
        yp = es.enter_context(tc.tile_pool(name="ysb", bufs=2))
        projp = es.enter_context(tc.tile_pool(name="pp", bufs=2, space="PSUM"))
        spsum = es.enter_context(tc.tile_pool(name="sp", bufs=3, space="PSUM"))
        opsum = es.enter_context(tc.tile_pool(name="op", bufs=2, space="PSUM"))
        wops = es.enter_context(tc.tile_pool(name="wop", bufs=1, space="PSUM"))

        # ---- one-time loads: consts, rope tables, all weights ----
        tri = constp.tile([128, 128], BF16, tag="tri")
        nc.sync.dma_start(tri[:], tridt[:])
        ones_c = constp.tile([128, 1], BF16, tag="onesc")
        nc.sync.dma_start(ones_c[:], onescol[:])
        cos_sb = constp.tile([HD, T], BF16, tag="cos")
        nc.sync.dma_start(cos_sb[:], cosdt[:])
        nsin_sb = constp.tile([HD, T], BF16, tag="nsin")
        nc.sync.dma_start(nsin_sb[:], nsindt[:])

        # x(0) and wk are emitted first (interleaved) so the first projection
        # wave starts as soon as its ct-tiles land, not after 12MB of weights.
        xb0 = []
        wk_sb, wq_sb, wv_sb, wo_sb = [], [], [], []
        for ct in range(NCT):
            t_ = xp.tile([128, TB], BF16, tag="xb", name=f"xb0_{ct}")
            nc.sync.dma_start(t_[:], xT[ct * 128 : (ct + 1) * 128, 0:TB])
            xb0.append(t_)
            w_ = wgtp.tile([128, NKV * HD], BF16, tag=f"wk{ct}")
            nc.sync.dma_start(w_[:], wkT[ct * 128 : (ct + 1) * 128, :])
            wk_sb.append(w_)
        for ct in range(NCT):
            t_ = wgtp.tile([128, NQH * HD], BF16, tag=f"wq{ct}")
            nc.sync.dma_start(t_[:], wqT[ct * 128 : (ct + 1) * 128, :])
            wq_sb.append(t_)
        for ct in range(NCT):
            t_ = wgtp.tile([128, NKV * HD], BF16, tag=f"wv{ct}")
            nc.sync.dma_start(t_[:], wvT[ct * 128 : (ct + 1) * 128, :])
            wv_sb.append(t_)
        for jh in range(NQH):
            t_ = wgtp.tile([128, C], BF16, tag=f"wo{jh}")
            nc.sync.dma_start(t_[:], woT[jh * 128 : (jh + 1) * 128, :])
            wo_sb.append(t_)

        kT = [perp.tile([HD, T], BF16, tag=f"kT{h}", name=f"kT{h}") for h in range(NKV)]
        vT = [perp.tile([128, NKV * HD], BF16, tag=f"v{i}", name=f"v{i}") for i in range(T // 128)]

        def _rope(dst, src_psum, tsl):
            """dst = src*cos + rot_half(src)*sin, src in [d, t] psum layout.

            rot_half(x)[d] = -x[d+64] for d<64, +x[d-64] for d>=64; the sign
            lives in nsin (rotated by 64 partitions on host) so both halves
            are plain multiplies with equal input base partitions.
            """
            cosb = cos_sb[:, tsl]
            nsinb = nsin_sb[:, tsl]
            t0 = tmpp.tile([HD, TB], BF16, tag="t0")
            nc.scalar.copy(t0[:], src_psum[:])  # frees the psum slot early
            tcc = tmpp.tile([HD, TB], BF16, tag="tc")
            nc.vector.tensor_mul(tcc[:], t0[:], cosb)
            t2 = tmpp.tile([HD, TB], BF16, tag="t2")
            nc.vector.tensor_mul(t2[0:64, :], t0[64:128, :], nsinb[64:128, :])
            nc.vector.tensor_mul(t2[64:128, :], t0[0:64, :], nsinb[0:64, :])
            nc.vector.scalar_tensor_tensor(dst, tcc[:], 1.0, t2[:], op0=MULT, op1=ADD)

        def load_block(tb):
            tsl = slice(tb * TB, (tb + 1) * TB)
            xb = []
            for ct in range(NCT):
                t_ = xp.tile([128, TB], BF16, tag="xb", name=f"xb{tb}_{ct}")
                nc.sync.dma_start(t_[:], xT[ct * 128 : (ct + 1) * 128, tsl])
                xb.append(t_)
            return xb

        def proj_block(tb, xb):
            tsl = slice(tb * TB, (tb + 1) * TB)
            # K projection (k^T layout [d, t]) + RoPE; one psum tile per wave
            # so the 2-slot psum pool genuinely double-buffers.
            for kw in range(4):
                kps = projp.tile([128, TB], F32, tag="pp", name=f"kps{tb}_{kw}")
                for ct in range(NCT):
                    nc.tensor.matmul(
                        kps[:],
                        wk_sb[ct][:, kw * 128 : (kw + 1) * 128],
                        xb[ct][:],
                        start=(ct == 0),
                        stop=(ct == NCT - 1),
                    )
                _rope(kT[kw][:, tsl], kps, tsl)

            # Q projection (q^T layout) + RoPE, eight 1-tile waves
            qts = []
            for wave in range(8):
                qps = projp.tile([128, TB], F32, tag="pp", name=f"qps{tb}_{wave}")
                for ct in range(NCT):
                    nc.tensor.matmul(
                        qps[:],
                        wq_sb[ct][:, wave * 128 : (wave + 1) * 128],
                        xb[ct][:],
                        start=(ct == 0),
                        stop=(ct == NCT - 1),
                    )
                qt = qp.tile([HD, TB], BF16, tag="qt", name=f"qt{tb}_{wave}")
                _rope(qt[:], qps, tsl)
                qts.append(qt)

            # V projection in [t, d] layout (x slice is the stationary side)
            for vw in range(4):
                vps = projp.tile([128, NKV * HD], F32, tag="pp", name=f"vps{tb}_{vw}")
                for ct in range(NCT):
                    nc.tensor.matmul(
                        vps[:],
                        xb[ct][:, vw * 128 : (vw + 1) * 128],
                        wv_sb[ct][:],
                        start=(ct == 0),
                        stop=(ct == NCT - 1),
                    )
                nc.scalar.copy(vT[4 * tb + vw][:], vps[:])
            return qts

        def attention_block(tb, qts):
            ktmax = 4 * tb + 4
            outs = []
            for h in range(NQH):
                hv = h // 2
                ops_ = opsum.tile([HD, TB], F32, tag="op", name=f"aop{tb}_{h}")
                den = opsum.tile([1, TB], F32, tag="op", name=f"den{tb}_{h}")
                exs = [None] * ktmax

                def emit_score(kt):
                    m = kt - 4 * tb
                    lo = 128 * max(m, 0)  # first causally-visible q column
                    sps = spsum.tile([128, TB], F32, tag="sp")
                    nc.tensor.matmul(
                        sps[:, lo:TB],
                        kT[hv][:, kt * 128 : (kt + 1) * 128],
                        qts[h][:, lo:TB],
                        start=True,
                        stop=True,
                    )
                    ex = expp.tile([128, TB], BF16, tag="exps")
                    nc.scalar.activation(ex[:, lo:TB], sps[:, lo:TB], EXP, scale=SCALE)
                    if m >= 0:
                        # triangular mask on the diagonal 128x128 sub-tile
                        nc.vector.tensor_mul(
                            ex[:, lo : lo + 128],
                            ex[:, lo : lo + 128],
                            tri[:],
                        )
                    exs[kt] = ex

                def emit_acc(kt):
                    m = kt - 4 * tb
                    lo = 128 * max(m, 0)
                    ex = exs[kt]
                    # denominator: accumulate ones.T @ ex on the PE in psum
                    nc.tensor.matmul(
                        den[0:1, lo:TB],
                        ones_c[:],
                        ex[:, lo:TB],
                        start=(kt == 0),
                        stop=(kt == ktmax - 1),
                    )
                    nc.tensor.matmul(
                        ops_[:, lo:TB],
                        vT[kt][:, hv * 128 : (hv + 1) * 128],
                        ex[:, lo:TB],
                        start=(kt == 0),
                        stop=(kt == ktmax - 1),
                    )

                # 2-step skew: score/exp run two kt ahead of den/out so the
                # accumulating matmuls never wait on the ACT exp stream.
                for kt in range(ktmax):
                    emit_score(kt)
                    if kt >= 2:
                        emit_acc(kt - 2)
                emit_acc(ktmax - 2) if ktmax >= 2 else None
                emit_acc(ktmax - 1)
                # single-op approx reciprocal (~18 bits, plenty), then
                # partition-broadcast on the otherwise idle GpSimd engine
                rec = smallp.tile([1, TB], F32, tag="rec")
                nc.vector.reciprocal_approx_fast(rec[:], den[0:1, :])
                bcs = smallp.tile([128, TB], F32, tag="bcs")
                nc.gpsimd.partition_broadcast(bcs[:], rec[0:1, :])
                ot = outp.tile([HD, TB], BF16, tag="ot")
                nc.vector.tensor_mul(ot[:], ops_[:], bcs[:])
                outs.append(ot)
            return outs

        def wo_block(tb, outs):
            tsl = slice(tb * TB, (tb + 1) * TB)
            for og in range(16):
                yps = wops.tile([128, TB], F32, tag="wop", name=f"yps{tb}_{og}")
                for jh in range(NQH):
                    nc.tensor.matmul(
                        yps[:],
                        wo_sb[jh][:, og * 128 : (og + 1) * 128],
                        outs[jh][:],
                        start=(jh == 0),
                        stop=(jh == NQH - 1),
                    )
                ysb = yp.tile([128, TB], F32, tag="ysb")
                nc.scalar.copy(ysb[:], yps[:])
                nc.gpsimd.dma_start(yT[og * 128 : (og + 1) * 128, tsl], ysb[:])

        # Software pipeline: attention/Wo of block tb-1 are emitted BEFORE
        # the projections of block tb; with run-ahead scheduling the dense
        # projection matmuls fill PE gaps in the ACT-gated attention phase.
        prev_qts = None
        for tb in range(NTB):
            xb = xb0 if tb == 0 else load_block(tb)
            if prev_qts is not None:
                outs = attention_block(tb - 1, prev_qts)
                wo_block(tb - 1, outs)
            prev_qts = proj_block(tb, xb)
        outs = attention_block(NTB - 1, prev_qts)
        wo_block(NTB - 1, outs)

    nc.compile()
    return nc


def _host_consts():
    inv_freq = 1.0 / (10000.0 ** (np.arange(0, HD, 2, dtype=np.float32) / HD))
    t = np.arange(T, dtype=np.float32)
    freqs = np.outer(t, inv_freq)  # [T, HD/2]
    freqs = np.repeat(freqs, 2, axis=-1)  # [T, HD]
    cos = np.cos(freqs).astype(np.float32).T.copy()  # [HD, T]
    sin = np.sin(freqs).astype(np.float32).T.copy()
    # rotated-by-64 signed sin table: row d holds the multiplier that pairs
    # with x[(d+64)%128]; rows 64..127 carry -sin[0:64], rows 0..63 +sin[64:128]
    nsin = np.empty_like(sin)
    nsin[0:64, :] = sin[64:128, :]
    nsin[64:128, :] = -sin[0:64, :]

    kp = np.arange(128)[:, None]
    qf = np.arange(128)[None, :]
    tri = (kp <= qf).astype(ml_dtypes.bfloat16)

    return {
        "cosdt": np.ascontiguousarray(cos.astype(ml_dtypes.bfloat16)),
        "nsindt": np.ascontiguousarray(nsin.astype(ml_dtypes.bfloat16)),
        "tridt": np.ascontiguousarray(tri),
        "onescol": np.ones((128, 1), dtype=ml_dtypes.bfloat16),
    }


_NC_CACHE = None


def _get_nc():
    global _NC_CACHE
    if _NC_CACHE is None:
        _NC_CACHE = build_nc()
    return _NC_CACHE


def kernel(x, Wq, Wk, Wv, Wo, _trace=False):
    x = np.asarray(x, dtype=np.float32)
    Wq = np.asarray(Wq, dtype=np.float32)
    Wk = np.asarray(Wk, dtype=np.float32)
    Wv = np.asarray(Wv, dtype=np.float32)
    Wo = np.asarray(Wo, dtype=np.float32)

    nc = _get_nc()
    consts = _host_consts()

    bf = ml_dtypes.bfloat16
    xTs = [np.ascontiguousarray(x[b].T.astype(bf)) for b in range(B)]
    wqTs = [np.ascontiguousarray(Wq[1024 * g : 1024 * (g + 1), :].T.astype(bf)) for g in range(2)]
    wkTs = [np.ascontiguousarray(Wk[512 * g : 512 * (g + 1), :].T.astype(bf)) for g in range(2)]
    wvTs = [np.ascontiguousarray(Wv[512 * g : 512 * (g + 1), :].T.astype(bf)) for g in range(2)]
    woTs = [np.ascontiguousarray(Wo[:, 1024 * g : 1024 * (g + 1)].T.astype(bf)) for g in range(2)]

    in_maps = []
    for c in range(8):
        b, g = c // 2, c % 2
        im = {
            "xT": xTs[b],
            "wqT": wqTs[g],
            "wkT": wkTs[g],
            "wvT": wvTs[g],
            "woT": woTs[g],
        }
        im.update(consts)
        in_maps.append(im)

    res = run_bass_kernel_spmd(nc, in_maps, core_ids=list(range(8)), trace=_trace)

    y = np.empty((B, T, C), dtype=np.float32)
    for b in range(B):
        y[b] = (res.results[2 * b]["yT"] + res.results[2 * b + 1]["yT"]).T
    if _trace:
        return y, res
    return y
